# revision 62
# baseline (speedup 1.0000x reference)
"""AttentionDTI on 8 Trainium2 NeuronCores — pure data-parallel over batch.

Strategy
--------
B=8 batches -> 1 batch per core (SPMD, no collectives). All parameters are
replicated; tokens are sharded along batch. The reference materializes the
(B, 85, 979, 160) pairwise tensor in HBM and applies a 160x160 linear to every
grid cell; since mean() commutes with the linear map, we only ever need
  Sd[i, c]  = sum_j relu(d_att[i, c] + p_att[j, c])     (row sums)
  Sp[j, c]  = sum_i relu(d_att[i, c] + p_att[j, c])     (col sums)
computed tile-by-tile in SBUF (the grid never touches HBM), followed by the
Wa linear + sigmoid on the tiny (85+979, 160) results.

Grid: channels live on partitions (chunk c0 = 0:128, c1 = 128:160 packed four
i-values per tile in 22-column blocks), protein positions on the free axis.
Units are produced on two engines:

* DVE (most units): one `tensor_scalar` per unit in the max-form
      h' = max(p_att, -d_att[i])  (= relu(d+p) - d)
  which runs in the 4x_2p DVE perf mode (~315 ns/unit vs ~1080 for
  scalar_tensor_tensor, which supports no fast modes), with the free-axis
  add-reduce accumulator giving the raw Sd column. The missing linear terms
  are restored afterwards: Sd += 979*d (folded into the bf16 conversion) and
  Sp += sum_i d  (folded as a per-partition bias into the PSUM->SBUF copy).
* ACT (trailing c0 units): relu-form activation with per-partition bias and
  free-axis accumulator (exact Sd), writing h in fp8e4 into pair buffers;
  each pair is consumed by one DoubleRow matmul (2 rows/cycle) so the PE-side
  Sp accumulation for these units costs 4x less than bf16. Pair matmuls are
  emitted one pair late so they never head-of-line-block the PE queue.
* a few DVE units' Sp accumulation rides the otherwise-idle Pool engine into
  an SBUF accumulator (merged during the final Sp evacuation), with enough
  tile buffers that the slow Pool never back-pressures the DVE.

Engine assignment is contiguous in i (DVE: i < XD0 plus all packed c1 units,
ACT: i >= XD0) so the Sp correction is one tensor_reduce over a slice. The
two unit streams are merged by proportional pacing so both engines finish
together; most of the drug-side attention output (catt + its Sd fixup) is
emitted mid-grid once its operands are final. A burst of junk matmuls at
t~1us keeps the PE p-state ramp warm so the convolutions run at 2.4 GHz.

All conv / attention-linear matmuls run in bf16 (fp32 matmul is 4 cycles/row
on the PE; bf16 is 1). PSUM accumulation stays fp32. Small parameters are
packed host-side into a handful of row-grouped blobs so the whole kernel
issues ~16 DMAs (HWDGE descriptor generation is ~0.6us per DMA, serialized);
token DMAs go first, then a small embedding pack (so the one-hot embedding
matmuls start ~2.5us in), conv weights in dependency order, big MLP weights
last.
"""

import os
import sys

import numpy as np

for _p in ("/opt/trn_rl_repo", "/root/.axon_site/_ro/trn_rl_repo"):
    if os.path.isdir(_p) and _p not in sys.path:
        sys.path.append(_p)

import concourse.bass as bass  # noqa: E402,F401
import concourse.bacc as bacc  # noqa: E402
import concourse.mybir as mybir  # noqa: E402
import concourse.tile as tile  # noqa: E402
from concourse import bass_utils  # noqa: E402

AFT = mybir.ActivationFunctionType
ALU = mybir.AluOpType
DT = mybir.dt
F32 = DT.float32
I32 = DT.int32
F8 = DT.float8e4
AXX = mybir.AxisListType.X
DROW = mybir.MatmulPerfMode.DoubleRow

NCORES = 8
B, LD, LP, DIM, CV = 8, 100, 1000, 64, 40
C = 4 * CV  # 160
DL1, DL2, DL3 = 97, 92, 85  # drug lengths after k=4,6,8 valid convs
PL1, PL2, PL3 = 997, 990, 979  # protein lengths after k=4,8,12
PADV = -1.0  # d-bias for padded i-slots; |p_att| < 0.1 so max(p, 1) == 1
# exactly and the +d correction cancels it exactly.
NGRP = (DL3 + 3) // 4  # 22 packed groups for channels 128:160 (block layout)

R32 = DT.bfloat16  # PE operand dtype: 1 cycle/row. (float32r would
# match bf16 speed at fp32-read precision but trips walrus ISA checks
# in this toolchain; plain fp32 is 4 cycles/row => ~2.5x slower.)
GRID_DT = R32  # pairwise grid tiles are written pre-rounded for the PE
MM_DT = R32  # conv / attention-linear matmul operands likewise

XA0 = 23  # c0 units produced on ACT (fp8 pairs); DVE takes the rest
XPOOL = 14  # DVE c0 units whose Sp accumulation rides on the Pool engine
POOLSP = 4  # spacing (in c0d units) between Pool-lane units
JSPLIT = 0  # (net-negative in sim: extra op overheads beat earlier start)
NLEAD = 12  # c0 units emitted before the c1 block (cover c1 bias DMAs)
CATT_J1 = 48  # catt columns computable mid-grid (after c0d unit J1-1)

_TRACE = bool(int(os.environ.get("KERNEL_TRACE", "0")))
LAST_RESULT = None
_CACHE = {}

# (pack -> (rows, [(key, cols, to_bf16), ...]));  bf16 entries must be a
# contiguous prefix so one tensor_copy converts the whole region.
# Dict order == DMA issue order == dependency order of the early pipeline.
PACKS = {
    "pkA": (128, [("drug_emb", DIM, 1), ("prot_emb", DIM, 1), ("iota", 1, 0)]),
    "pk64": (64, [("dW1T", 4 * CV, 1), ("pW1T", 4 * CV, 1),
                  ("db1", 1, 0), ("pb1", 1, 0)]),
    "pk40": (40, [("dW2T", 6 * 2 * CV, 1), ("pW2T", 8 * 2 * CV, 1)]),
    "pk80": (80, [("dW3T", 8 * C, 1), ("pW3T", 12 * C, 1),
                  ("db2", 1, 0), ("pb2", 1, 0)]),
    "pk128": (128, [("ident", 128, 1), ("WdaT_c0", C, 1), ("WpaT_c0", C, 1),
                    ("WaT_c0", C, 1), ("ident4", 32, 1), ("WpaTq_c0", 128, 1),
                    ("db3_c0", 1, 0), ("pb3_c0", 1, 0), ("bda_c0", 1, 0),
                    ("bpa_c0", 1, 0), ("ba_c0", 1, 0), ("bpa_c1x4", 1, 0)]),
    "pk32": (32, [("WdaT_c1", C, 1), ("WpaT_c1", C, 1), ("WaT_c1", C, 1),
                  ("WpaTq_c1", 128, 1), ("db3_c1", 1, 0), ("pb3_c1", 1, 0),
                  ("bda_c1", 1, 0), ("bpa_c1", 1, 0), ("ba_c1", 1, 0)]),
    "pkrow": (1, [("fb1", 1024, 0), ("fb2", 1024, 0), ("fb3", 512, 0),
                  ("fb4", 1, 0)]),
}


def _pack_offsets(pack):
    rows, entries = PACKS[pack]
    off, out = 0, {}
    for key, cols, bf in entries:
        out[key] = (off, cols, bf)
        off += cols
    return rows, off, out


def _jtiles(n, step=512):
    return [(o, min(step, n - o)) for o in range(0, n, step)]


def _mchunks(n):
    return [(o, min(128, n - o)) for o in range(0, n, 128)]


# --------------------------------------------------------------------------
# host-side parameter packing (pure marshalling, replicated to all cores)
# --------------------------------------------------------------------------

def _prep_shared(inp):
    def f32(x):
        return np.ascontiguousarray(np.asarray(x), dtype=np.float32)

    def convT(w):  # (co, ci, k) -> (ci, k*co) with [:, k0*co:(k0+1)*co] = tap k0
        co, ci, k = w.shape
        return np.ascontiguousarray(f32(w).transpose(1, 2, 0).reshape(ci, k * co))

    WdaT, WpaT, WaT = f32(inp["Wda"]).T, f32(inp["Wpa"]).T, f32(inp["Wa"]).T
    src = {
        "ident": np.eye(128, dtype=np.float32),
        "ident4": np.tile(np.eye(32, dtype=np.float32), (4, 1)),
        "iota": np.arange(128, dtype=np.float32).reshape(128, 1),
        "drug_emb": f32(inp["drug_emb"]), "prot_emb": f32(inp["prot_emb"]),
        "dW1T": convT(inp["dW1"]), "dW2T": convT(inp["dW2"]), "dW3T": convT(inp["dW3"]),
        "pW1T": convT(inp["pW1"]), "pW2T": convT(inp["pW2"]), "pW3T": convT(inp["pW3"]),
        "WdaT_c0": WdaT[0:128], "WdaT_c1": WdaT[128:C],
        "WpaT_c0": WpaT[0:128], "WpaT_c1": WpaT[128:C],
        "WaT_c0": WaT[0:128], "WaT_c1": WaT[128:C],
        # c1 columns of Wpa replicated 4x so the protein c1 attention comes
        # out of the matmul already block-broadcast across 128 partitions
        "WpaTq_c0": np.tile(WpaT[0:128, 128:C], (1, 4)),
        "WpaTq_c1": np.tile(WpaT[128:C, 128:C], (1, 4)),
        "bpa_c1x4": np.tile(f32(inp["bpa"]).reshape(-1, 1)[128:C], (4, 1)),
        "db1": f32(inp["db1"]).reshape(-1, 1), "db2": f32(inp["db2"]).reshape(-1, 1),
        "db3_c0": f32(inp["db3"]).reshape(-1, 1)[0:128],
        "db3_c1": f32(inp["db3"]).reshape(-1, 1)[128:C],
        "pb1": f32(inp["pb1"]).reshape(-1, 1), "pb2": f32(inp["pb2"]).reshape(-1, 1),
        "pb3_c0": f32(inp["pb3"]).reshape(-1, 1)[0:128],
        "pb3_c1": f32(inp["pb3"]).reshape(-1, 1)[128:C],
        "bda_c0": f32(inp["bda"]).reshape(-1, 1)[0:128],
        "bda_c1": f32(inp["bda"]).reshape(-1, 1)[128:C],
        "bpa_c0": f32(inp["bpa"]).reshape(-1, 1)[0:128],
        "bpa_c1": f32(inp["bpa"]).reshape(-1, 1)[128:C],
        "ba_c0": f32(inp["ba"]).reshape(-1, 1)[0:128],
        "ba_c1": f32(inp["ba"]).reshape(-1, 1)[128:C],
        "fb1": f32(inp["fb1"]).reshape(1, -1), "fb2": f32(inp["fb2"]).reshape(1, -1),
        "fb3": f32(inp["fb3"]).reshape(1, -1), "fb4": f32(inp["fb4"]).reshape(1, -1),
    }
    d = {}
    for pack in PACKS:
        rows, tot, offs = _pack_offsets(pack)
        arr = np.zeros((rows, tot), np.float32)
        for key, (off, cols, _) in offs.items():
            a = src[key]
            arr[0:a.shape[0], off:off + cols] = a
        d[pack] = arr
    # fW1T packed as 4 column segments (128/32/128/32 rows)
    fW1T = f32(inp["fW1"]).T  # (320, 1024)
    f1 = np.zeros((128, 4096), np.float32)
    for s, (r0, rn) in enumerate([(0, 128), (128, 32), (160, 128), (288, 32)]):
        f1[0:rn, 1024 * s:1024 * (s + 1)] = fW1T[r0:r0 + rn]
    d["fW1P"] = f1

    def pmajor(wT):  # (K, M) -> (128, (K/128)*M): chunk c at cols [c*M:(c+1)*M]
        k, m = wT.shape
        return np.ascontiguousarray(
            wT.reshape(k // 128, 128, m).transpose(1, 0, 2).reshape(128, -1))

    d["fW2P"] = pmajor(f32(inp["fW2"]).T)
    d["fW3P"] = pmajor(f32(inp["fW3"]).T)
    d["fW4P"] = pmajor(f32(inp["fW4"]).T)
    return d


def _dram_specs():
    specs = {"tok": ((1, LP + LD), I32)}
    for pack in PACKS:
        rows, tot, _ = _pack_offsets(pack)
        specs[pack] = ((rows, tot), F32)
    specs["fW1P"] = ((128, 4096), F32)
    specs["fW2P"] = ((128, 8192), F32)
    specs["fW3P"] = ((128, 4096), F32)
    specs["fW4P"] = ((128, 4), F32)
    return specs


_DRAM_SPECS = _dram_specs()


# --------------------------------------------------------------------------
# device kernel
# --------------------------------------------------------------------------

def build(xa0=XA0, debug=False, opts=()):
    opts = set(opts)
    nc = bacc.Bacc("TRN2", target_bir_lowering=False, debug=debug,
                   num_devices=NCORES)
    dram = {}
    for name, (shape, dt_) in _DRAM_SPECS.items():
        dram[name] = nc.dram_tensor(name, list(shape), dt_,
                                    kind="ExternalInput").ap()
    out_dram = nc.dram_tensor("out", [1, 1], F32, kind="ExternalOutput").ap()

    with tile.TileContext(nc) as tc:
        with (
            tc.tile_pool(name="w", bufs=1) as wp,
            tc.tile_pool(name="s", bufs=1) as sp,
            tc.tile_pool(name="h", bufs=4) as hp,
            tc.tile_pool(name="hP", bufs=XPOOL) as hpP,
            tc.tile_pool(name="h2", bufs=2) as h2p,
            tc.tile_pool(name="h2b", bufs=2) as h2b,
            tc.tile_pool(name="ps", bufs=3, space="PSUM") as pp,
            tc.tile_pool(name="pg", bufs=1, space="PSUM") as pg,
        ):
            _body(nc, tc, wp, sp, hp, hpP, h2p, h2b, pp, pg, dram, out_dram,
                  xa0, opts)
    nc.compile()
    return nc


def _body(nc, tc, wp, sp, hp, hpP, h2p, h2b, pp, pg, dram, out_dram, xa0,
          opts):
    # ---- one token DMA (protein+drug concatenated host-side, broadcast
    # to max-vocab rows in the DMA itself) -------------------------------
    tokb = sp.tile([65, LP + LD], I32, tag="tokb")
    nc.sync.dma_start(tokb, dram["tok"].broadcast_to((65, LP + LD)))
    tokb_p = tokb[0:26, 0:LP]
    tokb_d = tokb[:, LP:LP + LD]

    # ---- packed parameter loads; matmul-consumed regions (the bf-marked
    # prefix of each pack) are rounded to bf16 by a DVE copy --------------
    pk_f32, pk_r, pk_rcols = {}, {}, {}
    for pack in PACKS:
        rows, tot, offs = _pack_offsets(pack)
        t = wp.tile([rows, tot], F32, tag=pack)
        nc.sync.dma_start(t, dram[pack])
        pk_f32[pack] = t
        rcols = sum(cols for _, (off, cols, bf) in offs.items() if bf)
        pk_rcols[pack] = rcols
        if rcols:
            pk_r[pack] = wp.tile([rows, rcols], R32,
                                 name=f"{pack}_r", tag=f"{pack}_r")

    def convert(pack, c0=0, c1=None):
        c1 = pk_rcols[pack] if c1 is None else c1
        nc.vector.tensor_copy(pk_r[pack][:, c0:c1], pk_f32[pack][:, c0:c1])

    convert("pkA")  # embedding tables first: shortest dep chain

    def r32(ap):
        return ap if ap.dtype == R32 else ap.bitcast(R32)

    def P(key, rows=None, bf=True):
        for pack in PACKS:
            prows, _, offs = _pack_offsets(pack)
            if key in offs:
                off, cols, isbf = offs[key]
                t = pk_r[pack] if (bf and isbf) else pk_f32[pack]
                return t[0:(rows or prows), off:off + cols]
        raise KeyError(key)

    ones1 = wp.tile([1, 1], F32, tag="ones1")
    nc.vector.memset(ones1, 1.0)
    # warm the sigmoid ACT-table set now (relu/copy/identity are in every
    # set, so no further table loads happen mid-kernel)
    actwarm = wp.tile([1, 1], F32, tag="actwarm")
    nc.scalar.activation(actwarm, ones1, AFT.Sigmoid)

    # PE p-state warmup: the PE only reaches 2.4 GHz after ~3us of
    # continuous execution (1.2 GHz before that). Keep it chewing junk
    # matmuls from t~0.5us so the first conv layers run at full clock.
    if "no_warm" not in opts:
        junk = wp.tile([128, 256], R32, tag="junk")
        nc.vector.memset(junk, 0.0)
        junkps = pg.tile([128, 256], F32, tag="junkps", padded_shape=[128, 512])
        for _ in range(24):
            nc.tensor.matmul(junkps, junk[:, 0:128], junk, start=True,
                             stop=True)

    # ---- embeddings via one-hot matmul --------------------------------
    def embed(tokb, vocab, length, emb_sb, name):
        oh = sp.tile([vocab, length], MM_DT, tag=f"oh_{name}")
        nc.vector.tensor_scalar(oh, tokb, P("iota", rows=vocab, bf=False),
                                None, ALU.is_equal)
        res = sp.tile([DIM, length], MM_DT, tag=f"e_{name}")
        for j0, jn in _jtiles(length):
            ps = pp.tile([DIM, jn], F32, tag="ps")
            nc.tensor.matmul(ps, r32(emb_sb), r32(oh[:, j0:j0 + jn]),
                             start=True, stop=True)
            nc.scalar.activation(res[:, j0:j0 + jn], ps, AFT.Copy)
        return res

    pe = embed(tokb_p, 26, LP, P("prot_emb", rows=26), "p")
    de = embed(tokb_d, 65, LD, P("drug_emb", rows=65), "d")

    convert("pk64")
    convert("pk40")
    convert("pk80")
    convert("pk128")
    convert("pk32")
    zeros = wp.tile([128, 512], GRID_DT, tag="zeros")
    nc.vector.memset(zeros, 0.0)
    ident = P("ident")
    ident4 = P("ident4")

    # ---- CNN stacks (conv as K shifted matmuls accumulated in PSUM) ----
    def conv(tag, x, wT, biases, cout, k, lout, jt_major=False):
        outs = [sp.tile([msz, lout], MM_DT, name=f"{tag}_{mo}", tag=f"{tag}_{mo}")
                for mo, msz in _mchunks(cout)]
        n_ev = 0
        loops = [(ci, jt) for jt in _jtiles(lout) for ci in range(len(outs))] \
            if jt_major else \
            [(ci, jt) for ci in range(len(outs)) for jt in _jtiles(lout)]
        for ci, (j0, jn) in loops:
            mo, msz = _mchunks(cout)[ci]
            o = outs[ci]
            ps = pp.tile([msz, jn], F32, tag="ps")
            for t in range(k):
                nc.tensor.matmul(ps, r32(wT[:, cout * t + mo: cout * t + mo + msz]),
                                 r32(x[:, j0 + t: j0 + t + jn]),
                                 start=(t == 0), stop=(t == k - 1))
            if n_ev % 2 == 0:
                nc.scalar.activation(o[:, j0:j0 + jn], ps, AFT.Relu,
                                     bias=biases[ci][0:msz])
            else:
                nc.vector.scalar_tensor_tensor(o[:, j0:j0 + jn], ps,
                                               biases[ci][0:msz], zeros[0:msz, 0:jn],
                                               ALU.add, ALU.max)
            n_ev += 1
        return outs

    pc1 = conv("pc1", pe, P("pW1T", rows=DIM), [P("pb1", bf=False)], CV, 4, PL1)[0]
    dc1 = conv("dc1", de, P("dW1T", rows=DIM), [P("db1", bf=False)], CV, 4, DL1)[0]
    pc2 = conv("pc2", pc1, P("pW2T"), [P("pb2", bf=False)], 2 * CV, 8, PL2)[0]
    dc2 = conv("dc2", dc1, P("dW2T"), [P("db2", bf=False)], 2 * CV, 6, DL2)[0]
    # the whole drug chain (incl. datt) runs before the big protein conv3 so
    # the c1 bias-packing DMAs land long before the grid needs them
    dc3 = conv("dc3", dc2, P("dW3T"), [P("db3_c0", bf=False), P("db3_c1", bf=False)],
               C, 8, DL3)

    # ---- attention linears --------------------------------------------
    att_ev = [0]  # Identity evacuations alternate ACT / DVE

    def att_evac(o_slice, ps, bias):
        att_ev[0] += 1
        if att_ev[0] % 2:
            nc.scalar.activation(o_slice, ps, AFT.Identity, bias=bias)
        else:
            nc.vector.tensor_scalar(o_slice, ps, bias, None, ALU.add)

    def att_linear(tag, wTk, biases, xs, length, out_dt, pad_cols=0,
                   order=(0, 1)):
        outs = [None, None]
        for ci in order:
            mo, msz = _mchunks(C)[ci]
            o = sp.tile([msz, length + pad_cols], out_dt, tag=f"{tag}_{mo}")
            if pad_cols:
                nc.vector.memset(o[:, length:length + pad_cols], PADV)
            for j0, jn in _jtiles(length):
                ps = pp.tile([msz, jn], F32, tag="ps")
                for kc in range(len(xs)):
                    nc.tensor.matmul(ps, r32(wTk[kc][:, mo:mo + msz]),
                                     r32(xs[kc][:, j0:j0 + jn]),
                                     start=(kc == 0), stop=(kc == len(xs) - 1))
                att_evac(o[:, j0:j0 + jn], ps, biases[ci])
            outs[ci] = o
        return outs

    WpaT = [P("WpaT_c0"), P("WpaT_c1")]
    WdaT = [P("WdaT_c0"), P("WdaT_c1")]
    WaT = [P("WaT_c0"), P("WaT_c1")]
    bpac = [P("bpa_c0", bf=False), P("bpa_c1", bf=False)]
    bdac = [P("bda_c0", bf=False), P("bda_c1", bf=False)]
    bac = [P("ba_c0", bf=False), P("ba_c1", bf=False)]

    datt = att_linear("datt", WdaT, bdac, dc3, DL3, F32, pad_cols=3, order=(1, 0))
    # block-packed per-partition bias for channels 128:160:
    #   dattb_pk[32a + p, g] = datt_b[p, 22a + g]   (i = 22a + g, 85..87 = PADV)
    dattb_pk = sp.tile([128, NGRP], F32, tag="dattb_pk")
    for a in range(4):
        nc.sync.dma_start(dattb_pk[32 * a:32 * a + 32, :],
                          datt[1][:, NGRP * a:NGRP * a + NGRP])

    pc3 = conv("pc3", pc2, P("pW3T"), [P("pb3_c0", bf=False), P("pb3_c1", bf=False)],
               C, 12, PL3, jt_major=True)
    # patt chunk 0 first: its first jtile unblocks the j-split lead units
    patt = att_linear("patt", WpaT, bpac, pc3, PL3, GRID_DT, order=(0,))
    # protein c1 attention computed directly in 4x block-replicated layout
    # (replicated Wpa columns), replacing 4 SBUF-SBUF broadcast DMAs
    patt_b4 = sp.tile([128, PL3], GRID_DT, tag="patt_b4")
    WpaQ = [P("WpaTq_c0"), P("WpaTq_c1")]
    bpaq = P("bpa_c1x4", bf=False)
    for j0, jn in _jtiles(PL3):
        ps = pp.tile([128, jn], F32, tag="ps")
        nc.tensor.matmul(ps, r32(WpaQ[0]), r32(pc3[0][:, j0:j0 + jn]),
                         start=True, stop=False)
        nc.tensor.matmul(ps, r32(WpaQ[1]), r32(pc3[1][:, j0:j0 + jn]),
                         start=False, stop=True)
        att_evac(patt_b4[:, j0:j0 + jn], ps, bpaq)

    # ---- the pairwise grid --------------------------------------------
    xd0 = DL3 - xa0  # c0 DVE range [0, xd0), ACT range [xd0, 85)
    # negated d-bias columns for the DVE max-form
    negd0 = sp.tile([128, DL3 + 3], F32, tag="negd0")
    nc.vector.tensor_scalar(negd0, datt[0], -1.0, None, ALU.mult)
    negd_pk = sp.tile([128, NGRP], F32, tag="negd_pk")
    nc.vector.tensor_scalar(negd_pk, dattb_pk, -1.0, None, ALU.mult)
    # Sp corrections (the DVE units' tiles are h - d, short a sum_i d term)
    D0 = sp.tile([128, 1], F32, tag="D0")
    if xd0:
        nc.vector.tensor_reduce(D0, datt[0][:, 0:xd0], AXX, ALU.add)
    else:
        nc.vector.memset(D0, 0.0)
    D1pk = sp.tile([128, 1], F32, tag="D1pk")
    nc.vector.tensor_reduce(D1pk, dattb_pk, AXX, ALU.add)

    # fp8 identity pair [I | I] for DoubleRow Sp accumulation of ACT pairs
    identf8 = wp.tile([128, 256], F8, tag="identf8")
    nc.vector.tensor_copy(identf8[:, 0:128], P("ident", bf=False))
    nc.vector.tensor_copy(identf8[:, 128:256], P("ident", bf=False))
    identf8r = identf8.rearrange("p (two m) -> p two m", two=2)

    sd_c0 = sp.tile([128, DL3], F32, tag="sd_c0")
    sd_pk = sp.tile([128, NGRP], F32, tag="sd_pk")
    sp_a = pg.tile([128, PL3], F32, tag="sp_a", padded_shape=[128, 1024])
    sp_b = pg.tile([32, PL3], F32, tag="sp_b", padded_shape=[32, 1024])

    # Pool-lane: a few DVE units' Sp accumulation rides the (otherwise idle)
    # Pool engine into an SBUF f32 accumulator, relieving the PE
    pool_set = set(range(JSPLIT + POOLSP - 1, xd0, POOLSP))
    while len(pool_set) > XPOOL:
        pool_set.discard(max(pool_set))
    if "no_pool" in opts:
        pool_set = set()
    sp_pool = sp.tile([128, PL3], F32, tag="sp_pool")
    if pool_set:
        nc.gpsimd.memset(sp_pool, 0.0)

    # emission order: a few c0 leads (cover the c1 bias DMAs), then the
    # whole c1 block (its consumers overlap the c0 bulk), then the rest of
    # c0; ACT units are merged in proportionally so both unit streams span
    # the full grid phase.
    streamD = ([("c0d", i) for i in range(min(NLEAD, xd0))]
               + [("c1", g) for g in range(NGRP)]
               + [("c0d", i) for i in range(min(NLEAD, xd0), xd0)])
    streamA = [("c0a", i) for i in range(xd0, DL3)]
    costD, costA = 315.0 * len(streamD), 1188.0 * len(streamA)
    merged, tD, tA, iD, iA = [], 0.0, 0.0, 0, 0
    while iD < len(streamD) or iA < len(streamA):
        fD = tD / costD if costD else 2.0
        fA = tA / costA if costA else 2.0
        if iA >= len(streamA) or (iD < len(streamD) and fD <= fA):
            merged.append(streamD[iD]); iD += 1; tD += 315.0
        else:
            merged.append(streamA[iA]); iA += 1; tA += 1188.0
    if "no_grid" in opts:
        merged = [("c1", 0), ("c0d", 0), ("c0a", DL3 - 1)]

    n_ev_c0 = xd0 - len(pool_set) + (xa0 + 1) // 2  # matmul events into sp_a
    n_ev_c1 = NGRP
    jsplit = min(JSPLIT, xd0)
    sd_h0 = sp.tile([128, max(jsplit, 1)], F32, tag="sd_h0")
    sd_h1 = sp.tile([128, max(jsplit, 1)], F32, tag="sd_h1")
    ev_r = [0, 0]  # per-jtile-region event counters for the sp_a group
    ev_c1 = 0

    def c0_flags(t):
        f, l = ev_r[t] == 0, ev_r[t] == n_ev_c0 - 1
        ev_r[t] += 1
        return f, l
    last_c1_idx = max((k for k, u in enumerate(merged) if u[0] == "c1"),
                      default=-1)
    # catt part 1 can run once all c0d units < CATT_J1 and all c1 are done
    cj1 = min(CATT_J1, xd0)
    catt_idx = max((k for k, u in enumerate(merged)
                    if u[0] == "c1" or (u[0] == "c0d" and u[1] < cj1)),
                   default=-1)
    # +10 units of slack so catt's PE matmuls never head-of-line-block the
    # PE queue while the sd_b unpack DMAs (4x ~0.6us HWDGE) are in flight
    catt_idx = min(catt_idx + 10, len(merged) - 1)
    pair = {"tile": None, "col": 0}

    sd_bm = sd_b = spb_sb = None

    def flush_pair():
        t, ncol = pair["tile"], pair["col"]
        pair["tile"], pair["col"] = None, 0
        if ncol == 2:
            h2r = t.rearrange("p (two n) -> p two n", two=2)
            for rt, (j0, jn) in enumerate(_jtiles(PL3)):
                first, last = c0_flags(rt)
                nc.tensor.matmul(sp_a[:, j0:j0 + jn], identf8r,
                                 h2r[:, :, j0:j0 + jn],
                                 start=first, stop=last, perf_mode=DROW)
        else:
            for rt, (j0, jn) in enumerate(_jtiles(PL3)):
                first, last = c0_flags(rt)
                nc.tensor.matmul(sp_a[:, j0:j0 + jn], identf8[:, 0:128],
                                 t[:, j0:j0 + jn], start=first, stop=last)


    def emit_unit(kind, q):
        nonlocal ev_c1
        if kind == "c0a":
            if pair["tile"] is None or pair["col"] == 2:
                if pair["col"] == 2:
                    # flush the PREVIOUS pair only now: ACT has just finished
                    # it, so its matmuls never head-of-line-block the PE queue
                    flush_pair()
                pair["tile"] = h2p.tile([128, 2 * PL3], F8, name="H2",
                                        tag="H2")
            t, col = pair["tile"], pair["col"]
            nc.scalar.activation(t[:, col * PL3:(col + 1) * PL3], patt[0],
                                 AFT.Relu, bias=datt[0][:, q:q + 1],
                                 accum_out=sd_c0[:, q:q + 1])
            pair["col"] = col + 1
            return
        c0 = kind == "c0d"
        if not c0:
            h = hp.tile([128, PL3], GRID_DT, name="H", tag="H")
            nc.vector.tensor_scalar(h, patt_b4, negd_pk[:, q:q + 1], None,
                                    ALU.max, ALU.add,
                                    accum_out=sd_pk[:, q:q + 1])
            first, last = ev_c1 == 0, ev_c1 == n_ev_c1 - 1
            ev_c1 += 1
            for j0, jn in _jtiles(PL3):
                nc.tensor.matmul(sp_b[:, j0:j0 + jn], r32(ident4),
                                 r32(h[:, j0:j0 + jn]), start=first, stop=last)
            return
        negb = negd0[:, q:q + 1]
        if q < jsplit:
            # lead units split per jtile: the first halves only need the
            # first patt jtile, so the grid starts during the patt tail
            for rt, (j0, jn) in enumerate(_jtiles(PL3)):
                hh = hp.tile([128, jn], GRID_DT, name="Hh", tag="Hh")
                nc.vector.tensor_scalar(hh, patt[0][:, j0:j0 + jn], negb,
                                        None, ALU.max, ALU.add,
                                        accum_out=(sd_h0 if rt == 0 else
                                                   sd_h1)[:, q:q + 1])
                first, last = c0_flags(rt)
                nc.tensor.matmul(sp_a[:, j0:j0 + jn], r32(ident), r32(hh),
                                 start=first, stop=last)
            if q == jsplit - 1:
                nc.vector.tensor_tensor(sd_c0[:, 0:jsplit],
                                        sd_h0[:, 0:jsplit],
                                        sd_h1[:, 0:jsplit], ALU.add)
            return
        sd_ap = sd_c0[:, q:q + 1]
        pooled = q in pool_set
        h = (hpP if pooled else hp).tile([128, PL3], GRID_DT,
                                         name="Hp" if pooled else "H",
                                         tag="Hp" if pooled else "H")
        nc.vector.tensor_scalar(h, patt[0], negb, None, ALU.max, ALU.add,
                                accum_out=sd_ap)
        if pooled:
            nc.gpsimd.tensor_tensor(sp_pool, sp_pool, h, ALU.add)
            return
        for rt, (j0, jn) in enumerate(_jtiles(PL3)):
            first, last = c0_flags(rt)
            nc.tensor.matmul(sp_a[:, j0:j0 + jn], r32(ident),
                             r32(h[:, j0:j0 + jn]), start=first, stop=last)

    def c1_post():
        # c1 done: fix Sd (+979*d, exact 0 for pad slots), unpack via 4
        # contiguous DMAs, fold the Sp deficit 4x, and evacuate Sp_b with
        # the deficit as bias so its tail consumers overlap the c0 bulk.
        nonlocal sd_bm, sd_b, spb_sb
        sd_pkf = sp.tile([128, NGRP], F32, tag="sd_pkf")
        nc.vector.scalar_tensor_tensor(sd_pkf, dattb_pk, float(PL3), sd_pk,
                                       ALU.mult, ALU.add)
        psD1 = pp.tile([32, 1], F32, tag="ps")
        nc.tensor.matmul(psD1, P("ident4", bf=False), D1pk,
                         start=True, stop=True)
        D1s = sp.tile([32, 1], F32, tag="D1s")
        nc.scalar.activation(D1s, psD1, AFT.Copy)
        spb_sb = sp.tile([32, PL3], R32, tag="spb_sb")
        for j0, jn in _jtiles(PL3):
            nc.scalar.activation(spb_sb[:, j0:j0 + jn], sp_b[:, j0:j0 + jn],
                                 AFT.Identity, bias=D1s)
        sd_b = sp.tile([32, NGRP * 4], F32, tag="sd_b")
        for a in range(4):
            nc.sync.dma_start(sd_b[:, NGRP * a:NGRP * a + NGRP],
                              sd_pkf[32 * a:32 * a + 32, :])
        sd_bm = sp.tile([32, DL3], R32, tag="sd_bm")
        nc.vector.tensor_copy(sd_bm, sd_b[:, 0:DL3])

    # ---- attention outputs: sigmoid(Wa @ mean + ba), split so most of it
    # runs mid-grid (columns < cj1 as soon as their units are done) -------
    sd_c0m = sp.tile([128, DL3], R32, tag="sd_c0m")
    catt_t = [sp.tile([msz, DL3], F32, name=f"catt_{mo}", tag=f"catt_{mo}")
              for mo, msz in _mchunks(C)]

    def emit_catt(lo, hi):
        if lo < xd0:  # +979*d fixup on the DVE (max-form) range
            e = min(hi, xd0)
            nc.vector.scalar_tensor_tensor(sd_c0m[:, lo:e], datt[0][:, lo:e],
                                           float(PL3), sd_c0[:, lo:e],
                                           ALU.mult, ALU.add)
        if hi > xd0:
            s = max(lo, xd0)
            nc.vector.tensor_copy(sd_c0m[:, s:hi], sd_c0[:, s:hi])
        for ci, (mo, msz) in enumerate(_mchunks(C)):
            ps = pp.tile([msz, hi - lo], F32, tag="ps")
            nc.tensor.matmul(ps, r32(WaT[1][:, mo:mo + msz]),
                             r32(sd_bm[:, lo:hi]), start=True, stop=False)
            nc.tensor.matmul(ps, r32(WaT[0][:, mo:mo + msz]),
                             r32(sd_c0m[:, lo:hi]), start=False, stop=True)
            nc.scalar.activation(catt_t[ci][:, lo:hi], ps, AFT.Sigmoid,
                                 bias=bac[ci], scale=1.0 / PL3)

    for k, (kind, q) in enumerate(merged):
        emit_unit(kind, q)
        if k == last_c1_idx:
            c1_post()
        if k == catt_idx:
            emit_catt(0, cj1)
    if pair["tile"] is not None:  # odd ACT count: lone fp8 tile
        flush_pair()

    # Sp_a -> SBUF with the DVE-range deficit restored (+ Pool-lane partial)
    spa_sb = sp.tile([128, PL3], R32, tag="spa_sb")
    for j0, jn in _jtiles(PL3):
        if pool_set:
            nc.vector.scalar_tensor_tensor(spa_sb[:, j0:j0 + jn],
                                           sp_a[:, j0:j0 + jn], D0,
                                           sp_pool[:, j0:j0 + jn],
                                           ALU.add, ALU.add)
        else:
            nc.scalar.activation(spa_sb[:, j0:j0 + jn], sp_a[:, j0:j0 + jn],
                                 AFT.Identity, bias=D0)
    emit_catt(cj1, DL3)
    catt = catt_t
    # ---- protein tail: per (jtile, chunk): Wa matmuls -> sigmoid -> gate
    # -> partial max, fully pipelined across engines ----------------------
    jts = _jtiles(PL3)
    pvv = [sp.tile([msz, len(jts)], F32, name=f"pvv_{mo}", tag=f"pvv_{mo}")
           for mo, msz in _mchunks(C)]
    for t, (j0, jn) in enumerate(jts):
        for ci, (mo, msz) in enumerate(_mchunks(C)):
            ps = pp.tile([msz, jn], F32, tag="ps")
            nc.tensor.matmul(ps, r32(WaT[1][:, mo:mo + msz]),
                             r32(spb_sb[:, j0:j0 + jn]), start=True, stop=False)
            nc.tensor.matmul(ps, r32(WaT[0][:, mo:mo + msz]),
                             r32(spa_sb[:, j0:j0 + jn]), start=False, stop=True)
            pr = sp.tile([msz, jn], F32, name=f"pr_{mo}_{t}", tag="prt", bufs=2)
            nc.scalar.activation(pr, ps, AFT.Sigmoid, bias=bac[ci],
                                 scale=1.0 / DL3)
            g = sp.tile([msz, jn], F32, name=f"gp_{mo}_{t}", tag="gpt", bufs=2)
            nc.vector.scalar_tensor_tensor(g, pr, 0.5, pc3[ci][:, j0:j0 + jn],
                                           ALU.add, ALU.mult)
            nc.vector.tensor_reduce(pvv[ci][:, t:t + 1], g, AXX, ALU.max)
    pv = []
    for ci, (mo, msz) in enumerate(_mchunks(C)):
        v = sp.tile([msz, 1], F32, name=f"pv_{mo}", tag=f"pv_{mo}")
        nc.vector.tensor_reduce(v, pvv[ci], AXX, ALU.max)
        pv.append(v)

    # drug side is tiny: single-tile gate + max
    dv = []
    for ci, (mo, msz) in enumerate(_mchunks(C)):
        g = sp.tile([msz, DL3], F32, name=f"gd_{mo}", tag=f"gd_{mo}")
        nc.vector.scalar_tensor_tensor(g, catt[ci][:, 0:DL3], 0.5, dc3[ci],
                                       ALU.add, ALU.mult)
        v = sp.tile([msz, 1], F32, name=f"dv_{mo}", tag=f"dv_{mo}")
        nc.vector.tensor_reduce(v, g, AXX, ALU.max)
        dv.append(v)

    # ---- final MLP (weights DMA'd last; m on partitions, n=1 matvecs) --
    def wide_load(name, nchunks):
        shape, _ = _DRAM_SPECS[name]
        cols = shape[1] // nchunks
        t = wp.tile([128, shape[1]], F32, name=name, tag=name)
        nc.sync.dma_start(t, dram[name])
        return [t[:, cols * j:cols * (j + 1)] for j in range(nchunks)]

    fW1t = wp.tile([128, 4096], F32, tag="fW1P")
    nc.sync.dma_start(fW1t, dram["fW1P"])
    fW1k = [fW1t[0:128, 0:1024], fW1t[0:32, 1024:2048],
            fW1t[0:128, 2048:3072], fW1t[0:32, 3072:4096]]
    fW2k = wide_load("fW2P", 8)
    fW3k = wide_load("fW3P", 8)
    fW4k = wide_load("fW4P", 4)
    fb1, fb2, fb3 = P("fb1"), P("fb2"), P("fb3")
    fb4 = P("fb4")

    def dense(tag, xk, wk, bias_row, m, leaky):
        nm = m // 128
        ps = pp.tile([128, nm], F32, tag="ps")
        for mc in range(nm):
            for ci, (xv, wt) in enumerate(zip(xk, wk)):
                nc.tensor.matmul(ps[:, mc:mc + 1], wt[:, 128 * mc:128 * mc + 128],
                                 xv, start=(ci == 0), stop=False)
            nc.tensor.matmul(ps[:, mc:mc + 1], bias_row[0:1, 128 * mc:128 * mc + 128],
                             ones1, start=False, stop=True)
        yr = sp.tile([128, nm], F32, tag=f"yr{tag}")
        nc.vector.tensor_copy(yr, ps)
        if not leaky:
            return yr
        y = sp.tile([128, nm], F32, tag=f"y{tag}")
        nc.vector.scalar_tensor_tensor(y, yr, 0.01, yr, ALU.mult, ALU.max)
        return y

    y1 = dense("1", [dv[0], dv[1], pv[0], pv[1]], fW1k, fb1, 1024, True)
    y2 = dense("2", [y1[:, j:j + 1] for j in range(8)], fW2k, fb2, 1024, True)
    y3 = dense("3", [y2[:, j:j + 1] for j in range(8)], fW3k, fb3, 512, True)

    y4ps = pp.tile([1, 1], F32, tag="ps")
    for ci in range(4):
        nc.tensor.matmul(y4ps, fW4k[ci], y3[:, ci:ci + 1],
                         start=(ci == 0), stop=False)
    nc.tensor.matmul(y4ps, fb4, ones1, start=False, stop=True)
    res = sp.tile([1, 1], F32, tag="res")
    nc.vector.tensor_copy(res, y4ps)
    nc.sync.dma_start(out_dram, res)


# --------------------------------------------------------------------------
# entry point
# --------------------------------------------------------------------------

def _get_nc():
    key = ("v2", XA0)
    if key not in _CACHE:
        _CACHE[key] = build()
    return _CACHE[key]


def kernel(**inputs):
    global LAST_RESULT
    nc = _get_nc()
    shared = _prep_shared(inputs)
    drug = np.ascontiguousarray(np.asarray(inputs["drug"]), dtype=np.int32)
    protein = np.ascontiguousarray(np.asarray(inputs["protein"]), dtype=np.int32)
    tok = np.concatenate([protein, drug], axis=1)  # (B, LP+LD)
    in_maps = []
    for b in range(NCORES):
        m = dict(shared)
        m["tok"] = tok[b:b + 1]
        in_maps.append(m)
    res = bass_utils.run_bass_kernel_spmd(nc, in_maps, core_ids=list(range(NCORES)),
                                          trace=_TRACE)
    LAST_RESULT = res
    out = np.concatenate([res.results[b]["out"] for b in range(NCORES)], axis=0)
    return out.astype(np.float32)
